# revision 20
# baseline (speedup 1.0000x reference)
"""TRN2 Bass kernel for nn_DynamicWeightProjection.

Computes, for x = query_vec reshaped [B*T, D]:
    h   = gelu_exact(x @ W1)            W1 = dw1[:, 0, {0,2}, :]   -> 256 cols
    w_c = h_c @ qkw_c                   qkw_c = qkw[0, c] reshaped [128, 128]
    out = concat(rms(w_pre[:2]), rms(w_pre[2:])*s, tanh(x@dd)[0:32],
                 rms(w_post[:2]), rms(w_post[2:])*s, tanh(x@dd)[64:96])
Only C-splits {0, 2} and dd columns {0:32, 64:96} survive into the output,
so the fused first matmul needs just 320 of the 640 columns.

Strategy: 8-way data parallel over rows (B*T = 16384 -> 2048 rows/core).
All matmul operands are bf16 (halves HBM traffic vs fp32; rel err ~5e-3,
well under the 2e-2 gate). mm1 is X-STATIONARY: per 128-row block, the
x chunk [128d x 128rows] is the stationary operand and the fused weight
matrix streams 320 columns -> full 128-wide PE utilization (the previous
weights-stationary layout wasted half the array on the 64-wide dd group).
h lands in PSUM as [rows, 320]; gelu'd h is PE-transposed back to [k, rows]
for the small second matmul. The per-row-block tail (transpose, mm2, rms,
pack, store) is software-pipelined 1-2 blocks behind mm1 so the PE FIFO
never waits on the ACT/DVE chain.
"""
import numpy as np
from contextlib import ExitStack

import ml_dtypes

import concourse.bacc as bacc
import concourse.mybir as mybir
import concourse.tile as tile
from concourse.bass_utils import run_bass_kernel_spmd

AF = mybir.ActivationFunctionType
F32 = mybir.dt.float32
BF16 = mybir.dt.bfloat16

B, T, D = 4, 4096, 4096
NCORES = 8
ROWS = (B * T) // NCORES        # 2048 rows per core
RB = 128                        # rows per block (stationary-operand width)
NRB = ROWS // RB                # 16
DC = D // 128                   # 32 contraction chunks
WCOLS = 320                     # 256 w-cols (c=0,2) + 32 dd_pre + 32 dd_post
EPS = 1.1920929e-07


def build_nc(s2_scale=31250.0, s2_bias=EPS * 1e6, repeat=1, variant="full"):
    """Build the per-core SPMD program. s2_scale/s2_bias fold norm_scale into
    the w2 rms factor: rms(v)*s == 1/sqrt(ssum/(32 s^2) + eps/s^2).

    variant: "full" = real kernel; timing-ablation variants:
      "mm1"  = mm1 + gelu/tanh only (no transpose/mm2/rms/store)
      "resx" = full pipeline but x resident in SBUF (4 blocks cycled,
               wrong math for rb>=4 - timing only)
      "ldw2" = full pipeline, 16 distinct stationaries reused 2x each
               (half the LDWEIGHTS, wrong math - timing only)
      "peonly" = mm1 + TR + mm2 + minimal ACT evac, no DVE/rms/store
      "notr" = full but mm2 reads a const gT (no transposes, wrong math)
      "noout" = full minus the output DMA
      "actout" = full with output DMA issued from the scalar engine ring
      "dvediet" = full minus the rms math (keep wsb copy + store)
    """
    nc = bacc.Bacc("TRN2", target_bir_lowering=False, debug=False,
                   num_devices=NCORES, enable_partition_id=False)

    xt_in = nc.dram_tensor("xt", [NRB, 128, DC, RB], BF16, kind="ExternalInput")
    wall_in = nc.dram_tensor("wall", [128, DC, WCOLS], BF16, kind="ExternalInput")
    qkw_in = nc.dram_tensor("qkw2", [128, 2, 128], BF16, kind="ExternalInput")
    id_in = nc.dram_tensor("ident", [128, 128], BF16, kind="ExternalInput")
    out_d = nc.dram_tensor("out", [ROWS, WCOLS], F32, kind="ExternalOutput")

    with tile.TileContext(nc) as tc, ExitStack() as ctx:
        consts = ctx.enter_context(tc.tile_pool(name="consts", bufs=1))
        xfirst = ctx.enter_context(tc.tile_pool(name="xf", bufs=4))
        xpool = ctx.enter_context(tc.tile_pool(name="x", bufs=4))
        gpool = ctx.enter_context(tc.tile_pool(name="g", bufs=3))
        gtpool = ctx.enter_context(tc.tile_pool(name="gt", bufs=3))
        wpool = ctx.enter_context(tc.tile_pool(name="w", bufs=3))
        spool = ctx.enter_context(tc.tile_pool(name="s", bufs=3))
        papool = ctx.enter_context(tc.tile_pool(name="pack", bufs=8))
        ph = ctx.enter_context(tc.tile_pool(name="ph", bufs=2, space="PSUM"))
        pg = ctx.enter_context(tc.tile_pool(name="pg", bufs=2, space="PSUM"))
        pw = ctx.enter_context(tc.tile_pool(name="pw", bufs=2, space="PSUM"))

        wall_sb = consts.tile([128, DC, WCOLS], BF16)
        qkw_sb = consts.tile([128, 2, 128], BF16)
        id_sb = consts.tile([128, 128], BF16)
        bias1 = consts.tile([128, 1], F32)
        bias2 = consts.tile([128, 1], F32)
        nc.vector.memset(bias1[:], EPS)
        nc.vector.memset(bias2[:], s2_bias)
        gconst = None
        if variant == "notr":
            gconst = consts.tile([128, 256], BF16)
            nc.vector.memset(gconst[:], 0.5)

        # Prologue: interleave weight chunks with rb0's x pieces in
        # consumption order so the first matmuls wait on ~0.6 MiB only.
        first_tiles = []
        wall_groups = [(0, 4), (4, 8), (12, 8), (20, 12)]
        for k, (wg0, wglen) in enumerate(wall_groups):
            nc.sync.dma_start(wall_sb[:, wg0:wg0 + wglen, :],
                              wall_in[:, wg0:wg0 + wglen, :])
            xg0 = k * 8
            xf = xfirst.tile([128, 8, RB], BF16, tag="xf")
            nc.sync.dma_start(xf[:], xt_in[0, :, xg0:xg0 + 8, :])
            first_tiles.append((xg0, 8, xf))
        nc.sync.dma_start(qkw_sb[:], qkw_in[:])
        nc.sync.dma_start(id_sb[:], id_in[:])

        resx_tiles = []
        if variant == "resx":
            for rb in range(4):
                xt = xpool.tile([128, DC, RB], BF16, tag="xt")
                nc.sync.dma_start(xt[:], xt_in[rb])
                resx_tiles.append(xt)

        def emit_block(rep, rb):
            """mm1 + gelu/tanh for one 128-row block; returns (pa, pb)
            closures for the deferred transpose and mm2+rms stages."""
            if variant == "resx":
                tiles = [(0, DC, resx_tiles[rb % 4])]
            elif rep == 0 and rb == 0:
                tiles = first_tiles
            else:
                xt = xpool.tile([128, DC, RB], BF16, tag="xt")
                nc.sync.dma_start(xt[:], xt_in[rb])
                tiles = [(0, DC, xt)]

            h_ps = ph.tile([128, WCOLS], F32, tag="h")
            for g0, glen, xt in tiles:
                for l in range(glen):
                    dc = g0 + l
                    lhs = xt[:, l, :]
                    if variant == "ldw2":
                        lhs = xt[:, (l // 2) * 2, :]
                    nc.tensor.matmul(h_ps[:], lhs, wall_sb[:, dc, :],
                                     start=dc == 0, stop=dc == DC - 1)

            g_sb = gpool.tile([128, 256], BF16, tag="g")
            nc.scalar.activation(g_sb[:], h_ps[:, 0:256], AF.Gelu)
            pk = papool.tile([128, WCOLS], F32, tag="pk")
            nc.scalar.activation(pk[:, 128:160], h_ps[:, 256:288], AF.Tanh)
            nc.scalar.activation(pk[:, 288:320], h_ps[:, 288:320], AF.Tanh)

            state = {}

            def pa():
                if variant == "notr":
                    state["gT"] = None
                    return
                gT_ps = pg.tile([128, 256], BF16, tag="gt")
                nc.tensor.transpose(gT_ps[:, 0:128], g_sb[:, 0:128], id_sb[:])
                nc.tensor.transpose(gT_ps[:, 128:256], g_sb[:, 128:256], id_sb[:])
                gT_sb = gtpool.tile([128, 256], BF16, tag="gts")
                nc.scalar.activation(gT_sb[:], gT_ps[:], AF.Copy)
                state["gT"] = gT_sb

            def pb1():
                gT_sb = state["gT"]
                if gT_sb is None:
                    gT_sb = gconst
                w_ps = pw.tile([128, 256], F32, tag="w")
                nc.tensor.matmul(w_ps[:, 0:128], gT_sb[:, 0:128],
                                 qkw_sb[:, 0, :], start=True, stop=True)
                nc.tensor.matmul(w_ps[:, 128:256], gT_sb[:, 128:256],
                                 qkw_sb[:, 1, :], start=True, stop=True)
                wsb = wpool.tile([128, 256], F32, tag="wsb")
                nc.scalar.activation(wsb[:], w_ps[:], AF.Copy)
                state["wsb"] = wsb
                if variant in ("peonly", "dvediet"):
                    return
                sq = wpool.tile([128, 8, 32], F32, tag="sq")
                wv = wsb[:].rearrange("p (g m) -> p g m", m=32)
                nc.vector.tensor_mul(sq[:], wv, wv)
                ss = spool.tile([128, 8], F32, tag="ss")
                nc.vector.reduce_sum(ss[:], sq[:], axis=mybir.AxisListType.X)
                state["ss"] = ss

            def pb2():
                if variant == "peonly":
                    return
                wsb = state["wsb"]
                if variant != "dvediet":
                    ss = state["ss"]
                    fac = spool.tile([128, 8], F32, tag="fac")
                    ssv = ss[:].rearrange("p (c i) -> p c i", i=4)
                    facv = fac[:].rearrange("p (c i) -> p c i", i=4)
                    nc.scalar.activation(facv[:, :, 0:2], ssv[:, :, 0:2], AF.Sqrt,
                                         scale=1.0 / 32.0, bias=bias1[:, 0:1])
                    nc.scalar.activation(facv[:, :, 2:4], ssv[:, :, 2:4], AF.Sqrt,
                                         scale=s2_scale, bias=bias2[:, 0:1])
                    rfac = spool.tile([128, 8], F32, tag="rfac")
                    nc.vector.reciprocal(rfac[:], fac[:])

                    for c in range(2):
                        obase = 0 if c == 0 else 160
                        rbc = rfac[:, c * 4:(c + 1) * 4].unsqueeze(-1) \
                            .broadcast_to([128, 4, 32])
                        nc.vector.tensor_mul(
                            pk[:, obase:obase + 128].rearrange(
                                "p (i m) -> p i m", m=32),
                            wsb[:, c * 128:(c + 1) * 128].rearrange(
                                "p (i m) -> p i m", m=32),
                            rbc)
                if variant == "noout":
                    return
                # scalar-engine HWDGE ring: keeps the store off the sync ring
                # that streams the x tiles
                nc.scalar.dma_start(out_d[rb * RB:(rb + 1) * RB, :], pk[:])

            return pa, pb1, pb2

        # 4-deep software pipeline: at block k emit [TR(k-2) | mm2+ssum(k-3) |
        # rms+pack+store(k-4) | mm1(k)]. Deferred stages are emitted BEFORE
        # each mm1 so their ACT/DVE work sits ahead of gelu(k) (which blocks
        # on mm1(k)) in the strict per-engine FIFOs, and every cross-engine
        # dependency has >= 1 full block of slack -- neither ACT nor PE ever
        # waits on a same-iteration producer.
        K = repeat * NRB
        stages = {}
        for k in range(K):
            rep, rb = divmod(k, NRB)
            if variant == "mm1":
                emit_block(rep, rb)
                continue
            if k - 2 >= 0:
                stages[k - 2][0]()
            if k - 3 >= 0:
                stages[k - 3][1]()
            if k - 4 >= 0:
                stages[k - 4][2]()
                del stages[k - 4]
            stages[k] = emit_block(rep, rb)
        if variant != "mm1":
            stages[K - 2][0]()
            stages[K - 3][1]()
            stages[K - 4][2]()
            stages[K - 1][0]()
            stages[K - 2][1]()
            stages[K - 3][2]()
            stages[K - 1][1]()
            stages[K - 2][2]()
            stages[K - 1][2]()

    nc.compile()
    return nc


def host_prep(query_vec, dw1, qkw, dd, norm_scale):
    """Build per-core input maps (plus shared weight arrays), all bf16."""
    x = np.ascontiguousarray(query_vec.reshape(B * T, D)).astype(
        ml_dtypes.bfloat16)

    w1 = dw1[:, 0, 0, :]            # [D, 128]  pre_q
    w3 = dw1[:, 0, 2, :]            # [D, 128]  post_q
    ddp = dd[:, 0, 0:32]            # [D, 32]   pre_qdd
    ddq = dd[:, 0, 64:96]           # [D, 32]   post_qdd
    w_all = np.concatenate([w1, w3, ddp, ddq], axis=1)          # [D, 320]
    wall_h = np.ascontiguousarray(
        w_all.reshape(DC, 128, WCOLS).transpose(1, 0, 2)        # [128, DC, 320]
    ).astype(ml_dtypes.bfloat16)

    qkw2 = np.ascontiguousarray(
        qkw[0, [0, 2]].reshape(2, 128, 128).transpose(1, 0, 2)
    ).astype(ml_dtypes.bfloat16)                                 # [128, 2, 128]
    ident = np.eye(128, dtype=ml_dtypes.bfloat16)

    in_maps = []
    for c in range(NCORES):
        xc = x[c * ROWS:(c + 1) * ROWS]                         # [2048, 4096]
        xt = np.ascontiguousarray(
            xc.reshape(NRB, RB, DC, 128).transpose(0, 3, 2, 1))  # [16,128,32,128]
        in_maps.append({"xt": xt, "wall": wall_h, "qkw2": qkw2, "ident": ident})
    return in_maps


_NC_CACHE = {}


def get_nc(norm_scale):
    s = float(np.asarray(norm_scale).reshape(-1)[0])
    key = (s,)
    if key not in _NC_CACHE:
        _NC_CACHE[key] = build_nc(s2_scale=1.0 / (32.0 * s * s), s2_bias=EPS / (s * s))
    return _NC_CACHE[key]


def _run_device(nc, in_maps):
    res = run_bass_kernel_spmd(nc, in_maps, list(range(NCORES)))
    return np.concatenate([res.results[c]["out"] for c in range(NCORES)], axis=0)


def _run_subprocess(query_vec, dw1, qkw, dd, norm_scale):
    """Fresh-process fallback: a crashed/wedged device state lives in the
    axon client; a clean process (with core reset) usually recovers."""
    import os
    import subprocess
    import sys
    import tempfile
    d = tempfile.mkdtemp(prefix="dwp_kernel_")
    np.save(os.path.join(d, "query_vec.npy"), query_vec)
    np.save(os.path.join(d, "dw1.npy"), dw1)
    np.save(os.path.join(d, "qkw.npy"), qkw)
    np.save(os.path.join(d, "dd.npy"), dd)
    np.save(os.path.join(d, "norm_scale.npy"), norm_scale)
    prog = (
        "import numpy as np, importlib.util, sys\n"
        f"spec = importlib.util.spec_from_file_location('dwp_kernel', {__file__!r})\n"
        "m = importlib.util.module_from_spec(spec); spec.loader.exec_module(m)\n"
        f"d = {d!r}\n"
        "ins = {k: np.load(d + '/' + k + '.npy') for k in"
        " ('query_vec', 'dw1', 'qkw', 'dd', 'norm_scale')}\n"
        "out = m.kernel(_allow_subprocess=False, **ins)\n"
        "np.save(d + '/out.npy', out)\n"
    )
    env = dict(os.environ)
    env["NEURON_RT_RESET_CORES"] = "1"
    subprocess.run([sys.executable, "-c", prog], check=True, env=env,
                   timeout=1800)
    return np.load(os.path.join(d, "out.npy"))


def kernel(query_vec, dw1, qkw, dd, norm_scale, _allow_subprocess=True):
    nc = get_nc(norm_scale)
    in_maps = host_prep(query_vec, dw1, qkw, dd, norm_scale)
    try:
        out = _run_device(nc, in_maps)
    except Exception:
        if not _allow_subprocess:
            raise
        try:
            out = _run_device(nc, in_maps)       # in-process retry
        except Exception:
            out = _run_subprocess(query_vec, dw1, qkw, dd, norm_scale)
    return out.reshape(B, T, WCOLS)
